# revision 29
# baseline (speedup 1.0000x reference)
"""DenseCRFLoss on 8 Trainium2 NeuronCores.

Math: loss = -W/N * sum_k s_k^T K s_k per image, K[p,q] = exp(-0.5*||f_p-f_q||^2),
f = (x/50, y/50, rgb/15) on the 64x64 downsampled image, P=4096 pixels.

Device strategy (per core, SPMD-uniform program; data assignment differs per core):
  - K is symmetric with unit diagonal: only the strict upper triangle is computed
    (2x saving); the diagonal term sum(s^2) is a separate cheap reduction.
  - The PxP exp argument is produced directly by one bf16 matmul: the feature
    vectors are hi/lo bf16-split (fp32-accurate dot products) and the -0.5*|f|^2
    row/column terms are folded in as extra contraction rows, so the PSUM tile
    holds -0.5*d^2 exactly and ScalarE applies a plain exp (no bias plumbing).
  - The quadratic form uses a second tiny matmul per block (s rows as stationary)
    accumulating u = sum_rows s*E in PSUM per quad of 4 row-tiles, then one DVE
    mult+reduce per quad dots u with s_cols into per-quad partial sums.
  - Work unit: "quad" = 4 [128x512] blocks of one (image, column-chunk). Each
    image yields 36 quads (triangle); each of the 8 cores gets 18 quads of one
    image (2 cores per image). Every core runs the identical instruction stream.
  - Diagonal-straddling blocks skip the below-diagonal columns entirely and
    compute the diagonal 128x128 subtile UNMASKED with half-weighted s rows:
    by subtile symmetry 2*(total partial sums) equals the full quadratic form
    including the diagonal, so no masking and no separate diag term is needed.
"""

import numpy as np
import ml_dtypes

WEIGHT = 2e-9
SIGMA_RGB = 15.0
SIGMA_XY = 100.0
SCALE = 0.5

NQ = 18          # quads per core
NB = NQ * 4      # blocks per core
STRADDLE_SLOTS = (14, 15, 16, 17)  # quad slots holding diagonal-straddling quads
NROWS = 26       # contraction rows of the feature stack (24 + 2 bit-exp fold rows)

# Bit-trick exp (Schraudolph) offload: for blocks marked 'P' (GPSIMD/Pool) or
# 'D' (DVE), the G matmul folds +(BIT_SHIFT+BIT_DELTA) into the exp argument
# via fold rows 24/25, and the engine computes the bf16 bit pattern directly:
#   i16 = max(round(BIT_A * (g + 88 + delta)), 0);  bitcast -> bf16 ~ exp(g)
# max-clamp maps deep-negative args to +0.0. BIT_A*88 = 16250.45 embeds the
# bf16 exponent bias (127*128 = 16256), i.e. Schraudolph sigma = 0.0434.
BIT_A = 128.0 * float(np.log2(np.e))
BIT_SHIFT = 88.0
BIT_DELTA = -0.0111      # fine bias correction (calibrated vs reference)
# Pool/GPSIMD cannot read PSUM on this backend, so exp offload targets DVE
# only; the fused STT dot keeps DVE's total under the PE roofline.
POOL_LAST = set()               # multi-units whose last block goes to Pool
POOL_MID = set()                # multi-units whose middle block also goes
DVE_UNITS = set(range(18))      # multi-units whose last block -> DVE

# Straddle-quad packed G/E layout: block j covers chunk cols [128j:512], width
# 512-128j; packed into 3 PSUM banks as j0:[0:512] j1:[512:896] j3:[896:1024]
# j2:[1024:1280] (no block crosses a bank boundary).
_S_OFF = (0, 512, 1024, 896)
_S_W = (512, 384, 256, 128)

_bf16 = ml_dtypes.bfloat16

_PROGRAM_CACHE = {}


def _unit_structure():
    """Emission-ordered units of (quad_slot, j, col_off, engine) blocks.
    engine: 'A' = ACT exp, 'P' = Pool bit-exp, 'D' = DVE bit-exp."""
    fulls = [(i, j) for i in range(NQ) if i not in STRADDLE_SLOTS
             for j in range(4)]
    funits = [[(fulls[k][0], fulls[k][1], 0, 'A')] for k in range(3)]
    multi = [fulls[g:g + 3] for g in range(3, len(fulls), 3)]
    for u, blocks in enumerate(multi):
        ent = []
        for k, (i, j) in enumerate(blocks):
            eng = 'A'
            if k == len(blocks) - 1:
                if u in POOL_LAST:
                    eng = 'P'
                elif u in DVE_UNITS:
                    eng = 'D'
            elif k == 1 and u in POOL_MID and len(blocks) == 3:
                eng = 'P'
            ent.append((i, j, 512 * k, eng))
        funits.append(ent)
    sunits = [[(i, j, _S_OFF[j], 'A') for j in range(4)]
              for i in STRADDLE_SLOTS[:3]]
    s_last = [[(STRADDLE_SLOTS[-1], j, 0, 'A')] for j in range(4)]
    units = (funits[0:7] + sunits[0:1] + funits[7:12] + sunits[1:2]
             + funits[12:16] + sunits[2:3] + funits[16:] + s_last)
    assert sum(len(u) for u in units) == NB
    return units


def _offload_blocks():
    return {(i, j) for unit in _unit_structure()
            for (i, j, _off, eng) in unit if eng != 'A'}


def _build_program(reps=1, ablate=()):
    import concourse.bacc as bacc
    import concourse.tile as tile
    from concourse import mybir

    nc = bacc.Bacc("TRN2", target_bir_lowering=False)
    dt = mybir.dt

    feat = nc.dram_tensor("feat", [NROWS, 2, NQ, 512], dt.bfloat16, kind="ExternalInput")
    srows = nc.dram_tensor("srows", [128, NQ, 4, 4], dt.bfloat16, kind="ExternalInput")
    scols = nc.dram_tensor("scols", [2, NQ, 512], dt.bfloat16, kind="ExternalInput")
    out = nc.dram_tensor("out", [2, 20], dt.float32, kind="ExternalOutput")

    with tile.TileContext(nc) as tc:
        with (
            tc.tile_pool(name="consts", bufs=1) as consts,
            tc.tile_pool(name="gpsum", bufs=2, space="PSUM") as gpool,
            tc.tile_pool(name="gdpsum", bufs=2, space="PSUM") as gdpool,
            tc.tile_pool(name="upsum", bufs=2, space="PSUM") as upool,
            tc.tile_pool(name="epool", bufs=8) as epool,
            tc.tile_pool(name="edpool", bufs=6) as edpool,
            tc.tile_pool(name="scratch", bufs=6) as spool,
            tc.tile_pool(name="accp", bufs=4) as accp,
        ):
            # --- input DMAs (chunked so compute can start early) ---
            feat_sb = consts.tile([NROWS, 2, NQ, 512], dt.bfloat16)
            srows_sb = consts.tile([128, NQ, 4, 4], dt.bfloat16)
            scols_sb = consts.tile([2, NQ, 512], dt.bfloat16)
            stat_sb = feat_sb[:, 0]
            mov_sb = feat_sb[:, 1]

            # graded chunks, ordered by first use: tiny first chunk so
            # compute starts ASAP; straddle quads' chunks at their interleave
            chunks = [(0, 1), (1, 2), (2, 3), (3, 4), (14, 15), (4, 6), (6, 8),
                      (15, 16), (8, 10), (10, 12), (16, 17), (12, 14), (17, 18)]

            def feat_chunk(k, eng=None):
                lo, hi = chunks[k]
                (eng or nc.sync).dma_start(
                    out=feat_sb[:, :, lo:hi, :], in_=feat[:, :, lo:hi, :])

            # first-needed transfers on SP's HWDGE; later chunks go through
            # the GPSIMD SWDGE ring so they don't queue behind one HWDGE
            feat_chunk(0)
            feat_chunk(1)
            nc.sync.dma_start(out=srows_sb, in_=srows[:, :, :, :])
            nc.sync.dma_start(out=scols_sb, in_=scols[:, :, :])
            feat_chunk(2)
            for k in range(3, len(chunks)):
                feat_chunk(k, eng=nc.gpsimd)

            for _rep in range(reps):
                u_tiles = {}
                # cols 0..16: one dot per quad; the last straddle quad writes
                # three part-range dots to cols 17,19,18. Host sums all columns.
                acc_all = accp.tile([2, NQ + 2], dt.float32, name="acc_all")


                def dot_range(i, lo, hi, col):
                    # fused dot: acc_all[:, col] = sum(u[:, lo:hi] * s_cols)
                    scr = spool.tile([2, 512], dt.bfloat16, name="scr")
                    nc.vector.scalar_tensor_tensor(
                        out=scr[:, 0:hi - lo],
                        in0=u_tiles[i][:, lo:hi],
                        scalar=1.0,
                        in1=scols_sb[:, i, lo:hi],
                        op0=mybir.AluOpType.bypass,
                        op1=mybir.AluOpType.mult,
                        accum_out=acc_all[:, col:col + 1],
                    )

                def consume(unit, srcs):
                    """mask + u-matmuls + (on quad completion) the DVE dot."""
                    if "umm" in ablate or "dot" in ablate:
                        return
                    for (i, j, off), (e_tile, base) in zip(unit, srcs):
                        off = off - base
                        last = i == STRADDLE_SLOTS[-1]
                        if i in STRADDLE_SLOTS:
                            # Diagonal 128x128 subtile computed UNMASKED with
                            # half-weighted s rows: by subtile symmetry this
                            # contributes exactly (strict-upper) + diag/2, and
                            # 2*D' then equals 2*upper + diag — the full loss.
                            # For the last quad the group check is skipped so
                            # the incremental dots may read completed column
                            # ranges mid-group (per-element has_written makes
                            # this safe on silicon).
                            lo = 128 * j
                            nc.tensor.matmul(
                                out=u_tiles[i][:, lo:lo + 128],
                                lhsT=srows_sb[:, i, j, 2:4],
                                rhs=e_tile[:, off: off + 128],
                                start=(j == 0),
                                stop=(j == 3),
                                skip_group_check=last,
                            )
                            if j < 3:
                                nc.tensor.matmul(
                                    out=u_tiles[i][:, lo + 128:512],
                                    lhsT=srows_sb[:, i, j, 0:2],
                                    rhs=e_tile[:, off + 128: off + 512 - lo],
                                    start=False,
                                    stop=False,
                                    skip_group_check=last,
                                )
                        else:
                            nc.tensor.matmul(
                                out=u_tiles[i][:, 0:512],
                                lhsT=srows_sb[:, i, j, 0:2],
                                rhs=e_tile[:, off: off + 512],
                                start=(j == 0),
                                stop=(j == 3),
                            )
                        if i == STRADDLE_SLOTS[-1]:
                            # last quad: u[:, 0:128(j+1)] is final right after
                            # u-mm j, so split its dot — earlier ranges overlap
                            # the last exps and the tail chain is one short
                            # [2,128] fused dot.
                            if j == 1:
                                dot_range(i, 0, 256, 17)
                            elif j == 2:
                                dot_range(i, 256, 384, 19)
                            elif j == 3:
                                dot_range(i, 384, 512, 18)
                                u_tiles.pop(i)
                        elif j == 3:
                            dot_range(i, 0, 512, i)
                            u_tiles.pop(i)

                # Units: full quads stream as 3-block groups of [128,1536];
                # straddle quads 14-16 are packed [128,1280] units interleaved
                # among the full groups. The first three full blocks and the
                # last straddle quad run as single-block units: a short first
                # exp starts ACT sooner, and a short last exp plus incremental
                # dots shrink the serial tail. Blocks tagged P/D compute exp
                # as a bit-pattern tensor_scalar on Pool/DVE (see BIT_A).
                units = [[(i, j, off) for (i, j, off, _e) in unit]
                         for unit in _unit_structure()]
                engs = [[e for (_i, _j, _off, e) in unit]
                        for unit in _unit_structure()]

                # two-unit lag between production (G-mm + exp) and
                # consumption (u-mm/dot) so in-order PE/DVE queues never stall
                # upcoming G matmuls behind a dependency on a recent unit's E.
                pending = []
                for uk, unit in enumerate(units):
                    # Blocks split across two PSUM g-tiles so the two exp
                    # engines never read the same tile (a shared tile's
                    # cross-engine readers get serialized by tile sync):
                    # g_a [128,1024] holds the ACT blocks, g_d [128,512] the
                    # bit-exp block (or the straddle overflow block).
                    a_blocks = []   # (i, j, off, lo)
                    d_blocks = []
                    for bk, (i, j, off) in enumerate(unit):
                        if j == 0:
                            u_tiles[i] = upool.tile([2, 512], dt.float32, name="u_t")
                        lo = 128 * j if i in STRADDLE_SLOTS else 0
                        if engs[uk][bk] != 'A' or off >= 1024:
                            d_blocks.append((i, j, off, lo, engs[uk][bk]))
                        else:
                            a_blocks.append((i, j, off, lo, engs[uk][bk]))
                    assert len(d_blocks) <= 1
                    g_a = g_d = None
                    if a_blocks:
                        g_a = gpool.tile([128, 1024], dt.float32, name="g_a")
                    if d_blocks:
                        g_d = gdpool.tile([128, 512], dt.float32, name="g_d")
                    for i, j, off, lo, _e in a_blocks:
                        nc.tensor.matmul(
                            out=g_a[:, off: off + 512 - lo],
                            lhsT=stat_sb[:, i, j * 128:(j + 1) * 128],
                            rhs=mov_sb[:, i, lo:512],
                            start=True,
                            stop=True,
                        )
                    for i, j, off, lo, _e in d_blocks:
                        nc.tensor.matmul(
                            out=g_d[:, 0: 512 - lo],
                            lhsT=stat_sb[:, i, j * 128:(j + 1) * 128],
                            rhs=mov_sb[:, i, lo:512],
                            start=True,
                            stop=True,
                        )
                    src_map = {}
                    if a_blocks:
                        wa = max(off + 512 - lo for _i, _j, off, lo, _e in a_blocks)
                        e_a = epool.tile([128, 1024], dt.bfloat16, name="e_a")
                        nc.scalar.activation(
                            out=e_a[:, 0:wa],
                            in_=g_a[:, 0:wa],
                            func=mybir.ActivationFunctionType.Exp,
                        )
                        for i, j, off, lo, _e in a_blocks:
                            src_map[(i, j)] = (e_a, 0)
                    for i, j, off, lo, eng in d_blocks:
                        wd = 512 - lo
                        e_d = edpool.tile([128, 512], dt.bfloat16, name="e_d")
                        if eng == 'A':
                            nc.scalar.activation(
                                out=e_d[:, 0:wd],
                                in_=g_d[:, 0:wd],
                                func=mybir.ActivationFunctionType.Exp,
                            )
                        else:
                            ts = (nc.gpsimd if eng == 'P'
                                  else nc.vector).tensor_scalar
                            ts(
                                out=e_d[:, 0:wd].bitcast(dt.int16),
                                in0=g_d[:, 0:wd],
                                scalar1=BIT_A,
                                scalar2=0.0,
                                op0=mybir.AluOpType.mult,
                                op1=mybir.AluOpType.max,
                            )
                        src_map[(i, j)] = (e_d, off)
                    srcs = [src_map[(i, j)] for i, j, _off in unit]
                    pending.append((unit, srcs))
                    lag = 2 if uk < len(units) - 3 else 1
                    while len(pending) > lag:
                        consume(*pending.pop(0))
                for p in pending:
                    consume(*p)

                nc.sync.dma_start(out=out[:, :], in_=acc_all[:, :])

    nc.compile()
    return nc


def _get_program(reps=1):
    if reps not in _PROGRAM_CACHE:
        _PROGRAM_CACHE[reps] = _build_program(reps)
    return _PROGRAM_CACHE[reps]


def _quad_assignment():
    """Per-image quad lists for the two cores that share an image.
    Straddle quads must land on STRADDLE_SLOTS (the program masks those)."""
    full = [(c, q) for c in range(8) for q in range(c)]  # 28 quads
    stra = [(c, c) for c in range(8)]                    # 8 quads

    def arrange(fulls, stras):
        fi, si = iter(fulls), iter(stras)
        return [next(si) if s in STRADDLE_SLOTS else next(fi) for s in range(NQ)]

    even = arrange(full[0::2], stra[0:4])
    odd = arrange(full[1::2], stra[4:8])
    return even, odd


def _prepare_inputs(images, segmentations):
    """Host-side shard/pack: downsample, build bf16 hi/lo feature stacks,
    lay out per-core canonical quad arrays."""
    N = images.shape[0]
    assert images.shape == (4, 3, 128, 128) and segmentations.shape == (4, 2, 128, 128)

    # nearest resize (scale 0.5) == stride-2 subsample
    img = images[:, :, ::2, ::2].astype(np.float64)  # [4,3,64,64]

    # bilinear resize (scale 0.5, align_corners=False) == 2x2 average pooling;
    # mirror the reference's fp32 evaluation order exactly
    s = segmentations.astype(np.float32)
    t = s[:, :, 0::2, :] * np.float32(0.5) + s[:, :, 1::2, :] * np.float32(0.5)
    seg = t[:, :, :, 0::2] * np.float32(0.5) + t[:, :, :, 1::2] * np.float32(0.5)
    seg = seg.reshape(N, 2, 4096)  # [4,2,P] float32

    sxy = SIGMA_XY * SCALE
    yy, xx = np.meshgrid(np.arange(64.0), np.arange(64.0), indexing="ij")
    pos = np.stack([xx, yy], 0) / sxy  # [2,64,64]
    feats = np.concatenate(
        [np.broadcast_to(pos[None], (N, 2, 64, 64)), img / SIGMA_RGB], axis=1
    )  # [4,5,64,64]
    F = feats.reshape(N, 5, 4096)
    F = F - F.mean(axis=2, keepdims=True)  # translation-invariant; shrinks |f|
    b = -0.5 * (F * F).sum(axis=1)  # [4, P]

    def split(x):
        h = x.astype(_bf16).astype(np.float64)
        l = (x - h).astype(_bf16).astype(np.float64)
        return h, l

    Fh, Fl = split(F)          # [4,5,P] each
    Bh, Bl = split(b)          # [4,P]
    ones = np.ones((N, 1, 4096))
    zero = np.zeros((N, 1, 4096))

    # stat rows: Fh Fh Fl Fl | Bh Bl 1 1 | en en ;
    # mov rows:  Fh Fl Fh Fl | 1 1 Bh Bl | 88 delta
    # (en is a per-block bit-exp enable, patched per slot below)
    STAT = np.concatenate(
        [Fh, Fh, Fl, Fl, Bh[:, None], Bl[:, None], ones, ones, zero, zero],
        axis=1,
    ).astype(_bf16)  # [4, 26, P]
    MOV = np.concatenate(
        [Fh, Fl, Fh, Fl, ones, ones, Bh[:, None], Bl[:, None],
         np.full_like(ones, BIT_SHIFT), np.full_like(ones, BIT_DELTA)],
        axis=1,
    ).astype(_bf16)

    seg_bf = seg.astype(_bf16)
    seg_half = (seg * np.float32(0.5)).astype(_bf16)

    even, odd = _quad_assignment()
    offload = _offload_blocks()

    in_maps = []
    for core in range(8):
        im = core // 2
        quads = even if core % 2 == 0 else odd
        feat_arr = np.zeros((NROWS, 2, NQ, 512), _bf16)
        srows_arr = np.zeros((128, NQ, 4, 4), _bf16)
        scols_arr = np.zeros((2, NQ, 512), _bf16)
        for slot, (c, q) in enumerate(quads):
            feat_arr[:, 0, slot, :] = STAT[im][:, 512 * q: 512 * (q + 1)]
            feat_arr[:, 1, slot, :] = MOV[im][:, 512 * c: 512 * (c + 1)]
            for j in range(4):
                if (slot, j) in offload:
                    feat_arr[24:26, 0, slot, 128 * j:128 * (j + 1)] = 1.0
            for j in range(4):
                r = 4 * q + j
                srows_arr[:, slot, j, 0:2] = seg_bf[im][:, 128 * r: 128 * (r + 1)].T
                srows_arr[:, slot, j, 2:4] = seg_half[im][:, 128 * r: 128 * (r + 1)].T
            scols_arr[:, slot, :] = seg_bf[im][:, 512 * c: 512 * (c + 1)]
        in_maps.append(
            {
                "feat": np.ascontiguousarray(feat_arr),
                "srows": np.ascontiguousarray(srows_arr),
                "scols": np.ascontiguousarray(scols_arr),
            }
        )
    return in_maps


def _combine(outs, n_images=4):
    # diag-subtile half-weighting makes 2*sum(core partials) the full
    # quadratic form including the diagonal (see _build_program)
    off = sum(float(o["out"].sum(dtype=np.float64)) for o in outs)
    loss = -WEIGHT * 2.0 * off / n_images
    return np.array([loss], dtype=np.float32)


def kernel(images, segmentations):
    from concourse.bass_utils import run_bass_kernel_spmd

    in_maps = _prepare_inputs(np.asarray(images), np.asarray(segmentations))
    nc = _get_program(reps=1)
    last_err = None
    for _attempt in range(3):  # the NRT backend occasionally fails transiently
        try:
            res = run_bass_kernel_spmd(nc, in_maps, core_ids=list(range(8)))
            return _combine(res.results)
        except Exception as e:  # noqa: BLE001
            last_err = e
    raise last_err

